# revision 42
# baseline (speedup 1.0000x reference)
"""Trainium2 Bass kernel for nn_Attention_70557722739202.

Standard MHA block: qkv = x @ Wqkv.T + bqkv; attn = softmax(q k^T / 8);
out = (attn v) @ Wproj.T + bproj, with B=4, N=2048, C=768, H=12, hd=64
(ratio == 1 so the slimmable slicing is identity).

Sharding (8 cores): batch x head-group.  Core c handles batch c//2 and
heads [6*(c%2), 6*(c%2)+6).  Wqkv rows / Wproj cols are sharded by head;
each core emits a partial projection output [2048, 768] (bf16) and the
host sums the two partials per batch (+ bproj) in fp32.

Per-core dataflow (PE matmuls in bfloat16, fp32 PSUM; 1 moving row/cycle
at ~2.37GHz measured, ~100ns pipeline restart when adjacent matmuls
change geometry):
  - x.T, W slices DMA'd with input-channel on partitions (bf16 host-cast).
    x.T is split into four 512-col tiles so matmul DMA waits are
    per-slice.  DMA descriptor generation costs ~600ns per transfer,
    serial per issuing engine, so the input transfers split across the
    two HW-DGE engines (Sync + ScalarE) in need-order; engines only boot
    ~6.5us in, so the first transfer cannot land before ~10us.
  - warmup: v (seq-major, fused ones column for the softmax row-sums)
    for 4 seq chunks, then pair-0 q.T/k.T — the DMA-paced phase stays
    OUT of the pipelined loop (stalls inside it cascade through the psum
    rings and drop the PE out of its full-speed pstate; measured +55us).
  - attention, software-pipelined per (pair t, query-half j), chunk i =
    128 keys: PE sees [S x4][U(i-1) x4][filler]; exp runs out of PSUM
    with the 1/8 scale folded in.  ScalarE does ea ([128,1024], head A)
    + eb[:, :512]; the DVE computes eb[:, 512:] with a Schraudolph
    bit-trick exp (one tensor_scalar mult+add to int16, bitcast as bf16:
    bits = round(S*23.083 + 16250.5); HW rounds to nearest).  Offloading
    25% of the exp makes attention PE-paced instead of ScalarE-paced
    (2.14us/chunk exp vs ~1.95us/chunk PE; measured -10us vs all-ACT)
    and costs ~1.1% rms output error (total 7.9e-3 vs the 2e-2 gate).
    The S psum ring (2 slots) gives sb(i+1) a distance-1 WAR on the DVE
    read of sb(i-1), so everything queued on the DVE must stay promptly
    scheduled: qkv drains ride the ScalarE (Copy shares the Exp act
    table - no reload), and the normalize work is chopped into <=1.2us
    thunks.
  - U.T = [v | 1]^T expS.T accumulated over key chunks; rows 0..63 are
    the unnormalized attention output, row 64 the softmax denominator.
  - normalize per (t,j): U psum drains right at the boundary (recycles
    PSUM for the next group), then per-(head, 512-half) pieces — DMA the
    rowsum row to partition 0 (engines cannot shift partitions), DVE
    reciprocal_approx_fast, GpSimd partition_broadcast (the ONLY op type
    on GpSimd: mixing op types forces a ~7us Q7 LIBRARY_RELOAD each
    time), DVE multiply; head B goes through a bf16 staging tile + DMA
    partition shift to rows 64..127.  The pieces pop one-per-chunk from
    a deferred queue during the next group (the (2,1)-hosted projection
    fillers are emitted only after the thunk that writes the at-columns
    they read: dependency tracking is in-order, a reader emitted before
    its writer reads stale data).
  - projection: attnT chunks (stationary) x Wproj.T slices, PSUM K-accum,
    drained to bf16 and DMA'd out (host upcasts; halves output traffic).
    12 pieces (seq 0..1023) ride as (2,1) filler; the last 12 interleave
    with the final normalize at the tail, alternating ScalarE/DVE drains
    and pspool/upool psum slots so no single ring or engine paces them.

Fillers: QKV for pair t+1 inside pair t's groups, v(4..15) inside (0,0),
one per chunk.  epool 10 deep = 5 chunks of exp runahead.
Measured on HW: 429us fp32r -> 369 bf16 -> 322 baseline schedule ->
325-332 this layout (PE busy ~282us of which ~228us is the bf16
row-streaming floor; S/U geometry-switch restarts ~19us are inherent to
the 8-bank PSUM budget at QT=1024).
"""

import os
import sys

for _p in ("/opt/trn_rl_repo",):
    if os.path.isdir(_p) and _p not in sys.path:
        sys.path.insert(0, _p)

import numpy as np

import concourse.bacc as bacc
import concourse.mybir as mybir
import concourse.tile as tile
from concourse.bass_utils import run_bass_kernel_spmd

DIM = 768
NHEADS = 12
B, N = 4, 2048
HD = 64          # head dim
NCORES = 8
HPC = 6          # heads per core
PAIRS = 3        # head pairs per core
GPB = 2          # head groups per batch
CH = HPC * HD    # 384 output channels per core
SCALE = (DIM // NHEADS) ** -0.5
P = 128
QT = 1024        # query tile width (PSUM: 2 banks per S tile)
NKC = N // P     # 16 key chunks
KC = DIM // P    # 6 input-channel chunks
F32 = mybir.dt.float32
BF16 = mybir.dt.bfloat16
I16 = mybir.dt.int16
EXP = mybir.ActivationFunctionType.Exp

# Schraudolph bf16 exp: bits16 = round(S * A + B); bitcast -> bf16.
# A folds the 1/8 attention scale; B centers the piecewise-linear 2^f
# interpolation error band (+-3.3% max, 2.05% rms, measured on HW).
SCH_A = 128.0 / float(np.log(2.0)) * SCALE
SCH_B = 16256.0 - 5.5

# Offload 25% of the exp stream to the DVE via the bit-trick exp; False =
# all exp on the ScalarE (exact, baseline-style pacing).
SCHRAUDOLPH = True

_PROGRAMS = {}


def _emit(tc, xT_d, wqkT_d, wvT_d, bqk_d, bv_d, wpT_d, y_d, with_bias=True):
    nc = tc.nc

    from contextlib import ExitStack

    with ExitStack() as ctx:
        const = ctx.enter_context(tc.tile_pool(name="const", bufs=1))
        qkpool = ctx.enter_context(tc.tile_pool(name="qkpool", bufs=4))
        atpool = ctx.enter_context(tc.tile_pool(name="atpool", bufs=3))
        # epool depth is the exp runahead: 10 bufs = 5 chunks in flight so
        # the Scalar/Vector exp stream rides out the DVE's normalize-chain
        # convoys at (t,j) boundaries (the S psum ring has a distance-1 WAR
        # on the DVE Schraudolph read; a late DVE stalls the PE and drops
        # it out of its 2.4GHz pstate).
        epool = ctx.enter_context(tc.tile_pool(name="epool", bufs=10))
        rpool = ctx.enter_context(tc.tile_pool(name="rpool", bufs=4))
        rbpool = ctx.enter_context(tc.tile_pool(name="rbpool", bufs=4))
        uspool = ctx.enter_context(tc.tile_pool(name="uspool", bufs=4))
        ypool = ctx.enter_context(tc.tile_pool(name="ypool", bufs=3))
        pspool = ctx.enter_context(tc.tile_pool(name="pspool", bufs=2, space="PSUM"))
        upool = ctx.enter_context(tc.tile_pool(name="upool", bufs=2, space="PSUM"))

        # ---- resident inputs -------------------------------------------------
        # x.T split into 4 column-group tiles so matmul DMA waits are
        # per-group, not all-of-x.
        xts = [const.tile([P, KC, 512], BF16, name=f"xt{g}") for g in range(4)]
        wqk = const.tile([P, KC, 2 * CH], BF16)  # Wqk.T (in-ch on partitions)
        wv = const.tile([P, KC, CH], BF16)       # Wv.T
        wp = const.tile([P, PAIRS, DIM], BF16)   # Wproj.T slice (ch on part)
        bqk_sb = const.tile([1, 2 * CH], BF16)
        bv_sb = const.tile([1, CH], BF16)
        ones = const.tile([1, 512], BF16)
        v4 = const.tile([P, NKC, HPC * (HD + 1)], BF16)  # v + ones column

        # DMA descriptor generation costs ~600ns per transfer and is
        # serial per issuing engine — it was the real startup bottleneck
        # (33 transfers -> ~20us of descgen on Sync alone).  Split descgen
        # across the two HW-DGE engines (Sync + ScalarE, idle at t=0),
        # first-needed transfers first in BOTH queues: v(0..3) needs
        # xt0+wv, pair-0 qk then needs wqk+xt0..3.
        # need-order per engine queue: qk half nh reads xts[nh], emitted
        # ascending, so xt1/xt2 ride sync behind xt0 while wqk+xt3 ride
        # the scalar queue behind wv.
        for k in range(KC):
            nc.sync.dma_start(xts[0][:, k, :], xT_d[k * P:(k + 1) * P, 0:512])
            nc.scalar.dma_start(wqk[:, k, :], wqkT_d[k * P:(k + 1) * P, :])
        for k in range(KC):
            nc.scalar.dma_start(wv[:, k, :], wvT_d[k * P:(k + 1) * P, :])
        for g, eng in ((1, nc.sync), (2, nc.sync), (3, nc.sync)):
            csl = slice(g * 512, (g + 1) * 512)
            for k in range(KC):
                eng.dma_start(xts[g][:, k, :], xT_d[k * P:(k + 1) * P, csl])
        for t in range(PAIRS):
            nc.sync.dma_start(wp[:, t, :], wpT_d[t * P:(t + 1) * P, :])
        nc.sync.dma_start(bqk_sb[:], bqk_d[:])
        nc.sync.dma_start(bv_sb[:], bv_d[:])
        nc.vector.memset(ones[:], 1.0)
        # Dense memset to 1.0; the v drains below only overwrite columns
        # 0..63 of each 65-wide head block, leaving column 64 == 1.0 (the
        # fused softmax-rowsum column).
        nc.vector.memset(v4[:], 1.0)
        v4r = v4.rearrange("p n (h c) -> p n h c", c=HD + 1)

        qk_tiles = {}   # t -> (qt, kt)
        at_tiles = []

        def emit_qkv_half(t, part, nh):
            """One 512-col slice of pair t's q.T or k.T (nh in 0..3).
            PSUM is allocated and drained within the call."""
            if t not in qk_tiles:
                qt_ = qkpool.tile([P, N], BF16, tag="qk", name=f"qt{t}")
                kt_ = qkpool.tile([P, N], BF16, tag="qk", name=f"kt{t}")
                qk_tiles[t] = (qt_, kt_)
            qt_, kt_ = qk_tiles[t]
            colofs = t * P if part == "q" else CH + t * P
            dst = qt_ if part == "q" else kt_
            ps = pspool.tile([P, 512], F32, tag="s", name="qkps")
            for k in range(KC):
                nc.tensor.matmul(
                    ps[:],
                    lhsT=wqk[:, k, colofs:colofs + P],
                    rhs=xts[nh][:, k, :],
                    start=(k == 0),
                    stop=(k == KC - 1 and not with_bias),
                )
            if with_bias:
                nc.tensor.matmul(
                    ps[:],
                    lhsT=bqk_sb[:, colofs:colofs + P],
                    rhs=ones[:, 0:512],
                    start=False, stop=True,
                )
            if SCHRAUDOLPH:
                # drain on ScalarE (Copy shares the Exp table, no table
                # reload): measured slightly better than DVE when the ACT
                # has slack from the exp offload
                nc.scalar.copy(dst[:, nh * 512:(nh + 1) * 512], ps[:])
            else:
                nc.vector.tensor_copy(dst[:, nh * 512:(nh + 1) * 512], ps[:])

        def emit_v(s):
            """v for all 6 heads for sequence chunk s (with fused bias)."""
            g, cofs = divmod(s, 4)
            cofs *= P
            vps = pspool.tile([P, CH], F32, tag="s", name="vps")
            for k in range(KC):
                nc.tensor.matmul(
                    vps[:],
                    lhsT=xts[g][:, k, cofs:cofs + P],
                    rhs=wv[:, k, :],
                    start=(k == 0),
                    stop=(k == KC - 1 and not with_bias),
                )
            if with_bias:
                nc.tensor.matmul(
                    vps[:], lhsT=ones[:, 0:P], rhs=bv_sb[:],
                    start=False, stop=True,
                )
            # DVE, not ScalarE: the strided 3-D destination AP (64 of every
            # 65 columns) miscompiles/NaNs through the ACT copy path
            nc.vector.tensor_copy(
                v4r[:, s, :, 0:HD],
                vps.rearrange("p (h c) -> p h c", c=HD),
            )

        def emit_proj_piece(z, drain="dve", pool=None):
            """y.T piece: dims d*128..(d+1)*128, seq s4*512..(s4+1)*512.
            drain="act" uses the ScalarE for the PSUM drain (Copy shares
            the Exp activation table, so no table reload) — only safe at
            the tail where the exp backlog is gone, since the drain holds
            a pspool slot until it executes.  The tail also alternates
            psum between pspool and upool (attention done, U slots free)
            so the drain latency never gates the next piece's matmuls."""
            s4, dchunk = divmod(z, 6)
            # pool rings are per-tag: reuse the owning pool's existing tag
            # so no new PSUM ring is reserved
            yps = (pool or pspool).tile([P, 512], F32,
                                        tag="u" if pool is upool else "s",
                                        name="yps")
            for t in range(PAIRS):
                nc.tensor.matmul(
                    yps[:],
                    lhsT=wp[:, t, dchunk * P:(dchunk + 1) * P],
                    rhs=at_tiles[t][:, s4 * 512:(s4 + 1) * 512],
                    start=(t == 0), stop=(t == PAIRS - 1),
                )
            ysb = ypool.tile([P, 512], BF16, tag="y", name="ysb")
            if drain == "act":
                nc.scalar.copy(ysb[:], yps[:])
            else:
                nc.vector.tensor_copy(ysb[:], yps[:])
            nc.sync.dma_start(
                y_d[dchunk * P:(dchunk + 1) * P, s4 * 512:(s4 + 1) * 512],
                ysb[:],
            )

        def emit_proj_wide(dchunk):
            """1024-query-wide projection piece (queries 0..1023, one
            128-dim chunk): halves the pspool ring rotations and drains
            vs two 512-wide pieces — each rotation's WAR on the exp read
            of the slot 2 allocations back stalls the PE ~0.3-0.7us AND
            drops it to the 1.2GHz pstate for ~3us."""
            yps = pspool.tile([P, QT], F32, tag="s", name="yps")
            for t in range(PAIRS):
                for n in range(2):
                    nsl = slice(n * 512, (n + 1) * 512)
                    nc.tensor.matmul(
                        yps[:, nsl],
                        lhsT=wp[:, t, dchunk * P:(dchunk + 1) * P],
                        rhs=at_tiles[t][:, nsl],
                        start=(t == 0), stop=(t == PAIRS - 1),
                    )
            ysb = ypool.tile([P, QT], BF16, tag="y", name="ysb")
            nc.vector.tensor_copy(ysb[:], yps[:])
            nc.sync.dma_start(y_d[dchunk * P:(dchunk + 1) * P, 0:QT], ysb[:])

        def emit_u(t, ua, ub, ea, eb, i):
            # same-stationary matmuls adjacent: ua n0, ua n1, ub n0, ub n1
            for dst, e, h in ((ua, ea, 2 * t), (ub, eb, 2 * t + 1)):
                for n in range(QT // 512):
                    nsl = slice(n * 512, (n + 1) * 512)
                    nc.tensor.matmul(
                        dst[:, nsl], lhsT=v4r[:, i, h, :], rhs=e[:, nsl],
                        start=(i == 0), stop=(i == NKC - 1),
                    )

        def drain_u_half(u, name):
            """Drain one U psum tile's two halves to SBUF (recycles the
            PSUM slot for the next group's accumulation)."""
            out = []
            for h in range(2):
                us = uspool.tile([HD + 1, 512], F32, tag="us", name=name)
                nc.vector.tensor_copy(us[:], u[:, h * 512:(h + 1) * 512])
                out.append(us)
            return out

        def make_norm_thunks(t, j, ua, ub, at):
            """Boundary: drain ua now; the rest becomes small DVE thunks
            (<=1.2us each) popped one per chunk, so the DVE queue convoy
            never delays the Schraudolph exp (the S psum ring has a
            distance-1 WAR on it — a late DVE stalls the PE and drops it
            out of its 2.4GHz pstate).  Per-head 512-wide pieces keep each
            thunk's DVE time minimal."""
            st = {"usa": drain_u_half(ua, "usa")}
            thunks = []

            def t_drain_b():
                st["usb"] = drain_u_half(ub, "usb")

            def t_recip(h, side):
                u_half = st["usa" if side == "a" else "usb"][h]
                rs1 = rpool.tile([1, 512], F32, tag="rs", name="rs1")
                nc.sync.dma_start(rs1[:], u_half[HD:HD + 1, :])
                rc1 = rpool.tile([1, 512], F32, tag="r", name="rc1")
                nc.vector.reciprocal_approx_fast(rc1[:], rs1[:])
                rb1 = rbpool.tile([HD, 512], F32, tag="rb", name="rb1")
                nc.gpsimd.partition_broadcast(rb1[:], rc1[:])
                st[f"rb_{side}{h}"] = rb1

            def t_mul_a(h):
                jhsl = slice(j * QT + h * 512, j * QT + (h + 1) * 512)
                nc.vector.tensor_mul(at[0:HD, jhsl], st["usa"][h][0:HD, :],
                                     st[f"rb_a{h}"][:])

            def t_mul_b(h):
                jhsl = slice(j * QT + h * 512, j * QT + (h + 1) * 512)
                stg = uspool.tile([HD, 512], BF16, tag="stg", name="stg")
                nc.vector.tensor_mul(stg[:], st["usb"][h][0:HD, :],
                                     st[f"rb_b{h}"][:])
                nc.sync.dma_start(at[HD:P, jhsl], stg[:])

            thunks.append(t_drain_b)
            for h in range(2):
                thunks.append(lambda h=h: t_recip(h, "a"))
                thunks.append(lambda h=h: t_mul_a(h))
                thunks.append(lambda h=h: t_recip(h, "b"))
                thunks.append(lambda h=h: t_mul_b(h))
            return thunks


        # ---- warmup: v(0..3), then pair-0 q/k (dense PE stream; keeping
        # the DMA-gated qkv out of the pipelined loop — an early-start
        # variant with qk halves as (0,0) fillers measured 55us WORSE:
        # mid-pipeline DMA stalls cascade through the psum rings) ---------
        for s in range(4):
            emit_v(s)
        for part in ("q", "k"):
            for nh in range(4):
                emit_qkv_half(0, part, nh)

        # filler schedule: (t, j, i) -> [thunks] emitted after that chunk's U
        filler = {}

        def add_filler(key, fn):
            filler.setdefault(key, []).append(fn)

        for s in range(4, NKC):
            add_filler((0, 0, s - 2), lambda s=s: emit_v(s))
        # pair-1 halves ride in (0,1) (j=0 hosts the v fillers); pair-2
        # halves split across (1,0) and (1,1).
        qkv_spots = {
            0: [(1, i_) for i_ in (2, 3, 5, 6, 8, 9, 11, 12)],
            1: [(j_, i_) for j_ in range(2) for i_ in (2, 5, 8, 11)],
        }
        for t_ in range(PAIRS - 1):
            pieces = [(prt, nh) for prt in ("q", "k") for nh in range(4)]
            for (prt, nh), (j_, i_) in zip(pieces, qkv_spots[t_]):
                add_filler((t_, j_, i_),
                           lambda prt=prt, nh=nh, t_=t_:
                           emit_qkv_half(t_ + 1, prt, nh))
        # proj pieces must be EMITTED after the deferred normalize thunks
        # that write the at columns they read (in-order dependency
        # tracking: a reader emitted before its writer reads stale data):
        # with 9 thunks popped from chunk 1, (2,0)'s h1 muls finish by
        # chunk 9, so the six 1024-wide pieces sit at chunks 10..15.
        # dchunk 4,5 are held back to fill the tail's normalize-chain
        # PE gap (they only read at[:, 0:1024] — no tail dependency)
        for zz in range(4):
            add_filler((2, 1, 10 + zz), lambda zz=zz: emit_proj_wide(zz))

        # ---- attention: software-pipelined, continuous across j/pair
        # boundaries.  Per chunk the PE sees [S x4][U x4][filler]; the U
        # matmuls for chunk c are emitted AFTER chunk c+1's S + exp so the
        # exp stream never waits on PSUM recycling.
        pend = None   # (t, j, ua, ub, ea, eb, i, at)
        thunks = []   # deferred normalize pieces, one popped per chunk
        for t in range(PAIRS):
            qt_, kt_ = qk_tiles[t]
            at = atpool.tile([P, N], BF16, tag="at", name=f"at{t}")
            at_tiles.append(at)
            for j in range(N // QT):
                ua = upool.tile([HD + 1, QT], F32, tag="u", name="ua")
                ub = upool.tile([HD + 1, QT], F32, tag="u", name="ub")
                for i in range(NKC):
                    sa = pspool.tile([P, QT], F32, tag="s", name="sa")
                    sb = pspool.tile([P, QT], F32, tag="s", name="sb")
                    # same-stationary matmuls adjacent (measured ~5us
                    # better than alternating heads per n-slice)
                    for dst, row in ((sa, slice(0, HD)), (sb, slice(HD, P))):
                        for n in range(QT // 512):
                            qsl = slice(j * QT + n * 512, j * QT + (n + 1) * 512)
                            nc.tensor.matmul(
                                dst[:, n * 512:(n + 1) * 512],
                                lhsT=kt_[row, i * P:(i + 1) * P],
                                rhs=qt_[row, qsl],
                                start=True, stop=True,
                            )
                    ea = epool.tile([P, QT], BF16, tag="e", name="ea")
                    nc.scalar.activation(ea[:], sa[:], EXP, scale=SCALE)
                    eb = epool.tile([P, QT], BF16, tag="e", name="eb")
                    nc.scalar.activation(eb[:, 0:512], sb[:, 0:512], EXP,
                                         scale=SCALE)
                    if SCHRAUDOLPH:
                        # DVE Schraudolph exp for the second half of head B
                        # (25% of the exp stream off the ScalarE).
                        nc.vector.tensor_scalar(
                            eb.bitcast(I16)[:, 512:QT], sb[:, 512:QT],
                            SCH_A, SCH_B,
                            mybir.AluOpType.mult, mybir.AluOpType.add)
                    else:
                        nc.scalar.activation(eb[:, 512:QT], sb[:, 512:QT],
                                             EXP, scale=SCALE)
                    # pop a deferred normalize piece BEFORE the U emission
                    # so the prev group's ub drain precedes the next U
                    # matmuls in the DVE/PE queues
                    if thunks:
                        thunks.pop(0)()
                    if pend is not None:
                        pt, pj, pua, pub, pea, peb, pi, pat = pend
                        emit_u(pt, pua, pub, pea, peb, pi)
                        if pi == NKC - 1:
                            thunks = make_norm_thunks(pt, pj, pua, pub, pat)
                    pend = (t, j, ua, ub, ea, eb, i, at)
                    for fn in filler.get((t, j, i), ()):
                        fn()
        pt, pj, pua, pub, pea, peb, pi, pat = pend
        for th in thunks:
            th()

        # ---- tail: final U matmuls with the drains interleaved per
        # query-half (the h0 drains run while the PE streams h1), then a
        # latency-minimized per-head normalize chain feeding the last 12
        # projection pieces (seq 1024..2047).
        us_t = []
        for n in range(2):
            nsl = slice(n * 512, (n + 1) * 512)
            for u_, h_ in ((pua, 2 * pt), (pub, 2 * pt + 1)):
                nc.tensor.matmul(
                    u_[:, nsl], lhsT=v4r[:, pi, h_, :], rhs=(pea, peb)[h_ % 2][:, nsl],
                    start=False, stop=True,
                )
            usa_n = uspool.tile([HD + 1, 512], F32, tag="us", name="usa")
            nc.vector.tensor_copy(usa_n[:], pua[:, nsl])
            usb_n = uspool.tile([HD + 1, 512], F32, tag="us", name="usb")
            nc.vector.tensor_copy(usb_n[:], pub[:, nsl])
            us_t.append((usa_n, usb_n))
        # reserved wide pieces fill the PE while the first normalize chain
        # (drain -> rowsum DMA -> recip -> broadcast -> mul -> shift DMA,
        # ~5us serial) runs on DVE/GpSimd/DMA
        emit_proj_wide(4)
        emit_proj_wide(5)

        def tail_norm_head(u_half, rows, jhsl, to_b):
            """Per-head 512-wide chain (shorter serial latency than the
            merged 1024-wide variant used mid-kernel)."""
            rs1 = rpool.tile([1, 512], F32, tag="rs", name="rs1")
            nc.sync.dma_start(rs1[:], u_half[HD:HD + 1, :])
            rc1 = rpool.tile([1, 512], F32, tag="r", name="rc1")
            nc.vector.reciprocal_approx_fast(rc1[:], rs1[:])
            rb1 = rbpool.tile([HD, 512], F32, tag="rb", name="rb1")
            nc.gpsimd.partition_broadcast(rb1[:], rc1[:])
            if not to_b:
                nc.vector.tensor_mul(pat[0:HD, jhsl], u_half[0:HD, :], rb1[:])
            else:
                stg = uspool.tile([HD, 512], BF16, tag="stg", name="stg")
                nc.vector.tensor_mul(stg[:], u_half[0:HD, :], rb1[:])
                nc.sync.dma_start(pat[HD:P, jhsl], stg[:])

        for h in range(2):
            jhsl = slice(pj * QT + h * 512, pj * QT + (h + 1) * 512)
            tail_norm_head(us_t[h][0], slice(0, HD), jhsl, False)
            tail_norm_head(us_t[h][1], slice(HD, P), jhsl, True)
            for q, z in enumerate(range(12 + 6 * h, 18 + 6 * h)):
                emit_proj_piece(z, drain="act" if z % 2 else "dve",
                                pool=upool if q % 2 else pspool)


def build_program(with_bias=True):
    nc = bacc.Bacc(
        "TRN2", target_bir_lowering=False, debug=False, num_devices=NCORES
    )
    xT_d = nc.dram_tensor("xT", [DIM, N], BF16, kind="ExternalInput").ap()
    wqkT_d = nc.dram_tensor("wqkT", [DIM, 2 * CH], BF16, kind="ExternalInput").ap()
    wvT_d = nc.dram_tensor("wvT", [DIM, CH], BF16, kind="ExternalInput").ap()
    bqk_d = nc.dram_tensor("bqk", [1, 2 * CH], BF16, kind="ExternalInput").ap()
    bv_d = nc.dram_tensor("bv", [1, CH], BF16, kind="ExternalInput").ap()
    wpT_d = nc.dram_tensor("wpT", [CH, DIM], BF16, kind="ExternalInput").ap()
    # transposed output y.T [DIM, N] bf16; the host upcasts + transposes
    y_d = nc.dram_tensor("y", [DIM, N], BF16, kind="ExternalOutput").ap()
    with tile.TileContext(nc) as tc:
        _emit(tc, xT_d, wqkT_d, wvT_d, bqk_d, bv_d, wpT_d, y_d, with_bias)
    nc.compile()
    return nc


def get_program(with_bias=True):
    if with_bias not in _PROGRAMS:
        _PROGRAMS[with_bias] = build_program(with_bias)
    return _PROGRAMS[with_bias]


def make_in_maps(x, Wqkv, bqkv, Wproj):
    import ml_dtypes

    bf16 = ml_dtypes.bfloat16
    x = np.ascontiguousarray(np.asarray(x, np.float32))
    Wqkv = np.asarray(Wqkv, np.float32)
    bqkv = np.asarray(bqkv, np.float32)
    in_maps = []
    for c in range(NCORES):
        b, g = divmod(c, GPB)
        cs = slice(g * CH, (g + 1) * CH)
        wq = Wqkv[0 * DIM:1 * DIM][cs]
        wk = Wqkv[1 * DIM:2 * DIM][cs]
        wv_ = Wqkv[2 * DIM:3 * DIM][cs]
        in_maps.append({
            "xT": np.ascontiguousarray(x[b].T).astype(bf16),
            "wqkT": np.ascontiguousarray(
                np.concatenate([wq, wk], 0).T).astype(bf16),
            "wvT": np.ascontiguousarray(wv_.T).astype(bf16),
            "bqk": np.concatenate(
                [bqkv[0 * DIM:1 * DIM][cs], bqkv[1 * DIM:2 * DIM][cs]]
            )[None, :].astype(bf16),
            "bv": bqkv[2 * DIM:3 * DIM][cs][None, :].astype(bf16),
            "wpT": np.ascontiguousarray(
                np.asarray(Wproj, np.float32)[:, cs].T).astype(bf16),
        })
    return in_maps


def combine_outputs(per_core_y, bproj):
    bproj = np.asarray(bproj, np.float32)
    out = np.empty((B, N, DIM), np.float32)
    for b in range(B):
        out[b] = (np.asarray(per_core_y[GPB * b], np.float32)
                  + np.asarray(per_core_y[GPB * b + 1], np.float32)).T \
            + bproj[None, :]
    return out


def kernel(**inputs):
    ratio = int(np.asarray(inputs.get("ratio", 1)))
    assert ratio == 1, f"kernel specialized for ratio=1, got {ratio}"
    with_bias = bool(np.any(np.asarray(inputs["bqkv"], np.float32)))
    nc = get_program(with_bias)
    in_maps = make_in_maps(
        inputs["x"], inputs["Wqkv"], inputs["bqkv"], inputs["Wproj"]
    )
    res = run_bass_kernel_spmd(nc, in_maps, list(range(NCORES)))
    ys = [res.results[c]["y"] for c in range(NCORES)]
    return combine_outputs(ys, inputs["bproj"])


# revision 43
# speedup vs baseline: 1.0131x; 1.0131x over previous
"""Trainium2 Bass kernel for nn_Attention_70557722739202.

Standard MHA block: qkv = x @ Wqkv.T + bqkv; attn = softmax(q k^T / 8);
out = (attn v) @ Wproj.T + bproj, with B=4, N=2048, C=768, H=12, hd=64
(ratio == 1 so the slimmable slicing is identity).

Sharding (8 cores): batch x head-group.  Core c handles batch c//2 and
heads [6*(c%2), 6*(c%2)+6).  Wqkv rows / Wproj cols are sharded by head;
each core emits a partial projection output [2048, 768] (bf16) and the
host sums the two partials per batch (+ bproj) in fp32.

Per-core dataflow (PE matmuls in bfloat16, fp32 PSUM; 1 moving row/cycle
at ~2.37GHz measured, ~100ns pipeline restart when adjacent matmuls
change geometry):
  - x.T, W slices DMA'd with input-channel on partitions (bf16 host-cast).
    x.T is split into four 512-col tiles so matmul DMA waits are
    per-slice.  DMA descriptor generation costs ~600ns per transfer,
    serial per issuing engine, so the input transfers split across the
    two HW-DGE engines (Sync + ScalarE) in need-order; engines only boot
    ~6.5us in, so the first transfer cannot land before ~10us.
  - warmup: v (seq-major, fused ones column for the softmax row-sums)
    for 4 seq chunks, then pair-0 q.T/k.T — the DMA-paced phase stays
    OUT of the pipelined loop (stalls inside it cascade through the psum
    rings and drop the PE out of its full-speed pstate; measured +55us).
  - attention, software-pipelined per (pair t, query-half j), chunk i =
    128 keys: PE sees [S x4][U(i-1) x4][filler]; exp runs out of PSUM
    with the 1/8 scale folded in.  ScalarE does ea ([128,1024], head A)
    + eb[:, :512]; the DVE computes eb[:, 512:] with a Schraudolph
    bit-trick exp (one tensor_scalar mult+add to int16, bitcast as bf16:
    bits = round(S*23.083 + 16250.5); HW rounds to nearest).  Offloading
    25% of the exp makes attention PE-paced instead of ScalarE-paced
    (2.14us/chunk exp vs ~1.95us/chunk PE; measured -10us vs all-ACT)
    and costs ~1.1% rms output error (total 7.9e-3 vs the 2e-2 gate).
    The S psum ring (2 slots) gives sb(i+1) a distance-1 WAR on the DVE
    read of sb(i-1), so everything queued on the DVE must stay promptly
    scheduled: qkv drains ride the ScalarE (Copy shares the Exp act
    table - no reload), and the normalize work is chopped into <=1.2us
    thunks.
  - U.T = [v | 1]^T expS.T accumulated over key chunks; rows 0..63 are
    the unnormalized attention output, row 64 the softmax denominator.
  - normalize per (t,j): U psum drains right at the boundary (recycles
    PSUM for the next group), then per-(head, 512-half) pieces — DMA the
    rowsum row to partition 0 (engines cannot shift partitions), DVE
    reciprocal_approx_fast, GpSimd partition_broadcast (the ONLY op type
    on GpSimd: mixing op types forces a ~7us Q7 LIBRARY_RELOAD each
    time), DVE multiply; head B goes through a bf16 staging tile + DMA
    partition shift to rows 64..127.  The pieces pop one-per-chunk from
    a deferred queue during the next group (the (2,1)-hosted projection
    fillers are emitted only after the thunk that writes the at-columns
    they read: dependency tracking is in-order, a reader emitted before
    its writer reads stale data).
  - projection: attnT chunks (stationary) x Wproj.T slices, PSUM K-accum,
    drained to bf16 and DMA'd out (host upcasts; halves output traffic).
    12 pieces (seq 0..1023) ride as (2,1) filler; the last 12 interleave
    with the final normalize at the tail, alternating ScalarE/DVE drains
    and pspool/upool psum slots so no single ring or engine paces them.

Fillers: QKV for pair t+1 inside pair t's groups, v(4..15) inside (0,0),
one per chunk.  epool 10 deep = 5 chunks of exp runahead.
Measured on HW: 429us fp32r -> 369 bf16 -> 322 baseline schedule ->
325-332 this layout (PE busy ~282us of which ~228us is the bf16
row-streaming floor; S/U geometry-switch restarts ~19us are inherent to
the 8-bank PSUM budget at QT=1024).
"""

import os
import sys

for _p in ("/opt/trn_rl_repo",):
    if os.path.isdir(_p) and _p not in sys.path:
        sys.path.insert(0, _p)

import numpy as np

import concourse.bacc as bacc
import concourse.mybir as mybir
import concourse.tile as tile
from concourse.bass_utils import run_bass_kernel_spmd

DIM = 768
NHEADS = 12
B, N = 4, 2048
HD = 64          # head dim
NCORES = 8
HPC = 6          # heads per core
PAIRS = 3        # head pairs per core
GPB = 2          # head groups per batch
CH = HPC * HD    # 384 output channels per core
SCALE = (DIM // NHEADS) ** -0.5
P = 128
QT = 1024        # query tile width (PSUM: 2 banks per S tile)
NKC = N // P     # 16 key chunks
KC = DIM // P    # 6 input-channel chunks
F32 = mybir.dt.float32
BF16 = mybir.dt.bfloat16
I16 = mybir.dt.int16
EXP = mybir.ActivationFunctionType.Exp

# Schraudolph bf16 exp: bits16 = round(S * A + B); bitcast -> bf16.
# A folds the 1/8 attention scale; B centers the piecewise-linear 2^f
# interpolation error band (+-3.3% max, 2.05% rms, measured on HW).
SCH_A = 128.0 / float(np.log(2.0)) * SCALE
SCH_B = 16256.0 - 5.5

# Offload 25% of the exp stream to the DVE via the bit-trick exp; False =
# all exp on the ScalarE (exact, baseline-style pacing).
SCHRAUDOLPH = True

_PROGRAMS = {}


def _emit(tc, xT_d, wqkT_d, wvT_d, bqk_d, bv_d, wpT_d, y_d, with_bias=True):
    nc = tc.nc

    from contextlib import ExitStack

    with ExitStack() as ctx:
        const = ctx.enter_context(tc.tile_pool(name="const", bufs=1))
        qkpool = ctx.enter_context(tc.tile_pool(name="qkpool", bufs=4))
        atpool = ctx.enter_context(tc.tile_pool(name="atpool", bufs=3))
        # epool depth is the exp runahead: 10 bufs = 5 chunks in flight so
        # the Scalar/Vector exp stream rides out the DVE's normalize-chain
        # convoys at (t,j) boundaries (the S psum ring has a distance-1 WAR
        # on the DVE Schraudolph read; a late DVE stalls the PE and drops
        # it out of its 2.4GHz pstate).
        epool = ctx.enter_context(tc.tile_pool(name="epool", bufs=10))
        rpool = ctx.enter_context(tc.tile_pool(name="rpool", bufs=4))
        rbpool = ctx.enter_context(tc.tile_pool(name="rbpool", bufs=4))
        uspool = ctx.enter_context(tc.tile_pool(name="uspool", bufs=4))
        ypool = ctx.enter_context(tc.tile_pool(name="ypool", bufs=3))
        pspool = ctx.enter_context(tc.tile_pool(name="pspool", bufs=2, space="PSUM"))
        upool = ctx.enter_context(tc.tile_pool(name="upool", bufs=2, space="PSUM"))

        # ---- resident inputs -------------------------------------------------
        # x.T split into 4 column-group tiles so matmul DMA waits are
        # per-group, not all-of-x.
        xts = [const.tile([P, KC, 512], BF16, name=f"xt{g}") for g in range(4)]
        wqk = const.tile([P, KC, 2 * CH], BF16)  # Wqk.T (in-ch on partitions)
        wv = const.tile([P, KC, CH], BF16)       # Wv.T
        wp = const.tile([P, PAIRS, DIM], BF16)   # Wproj.T slice (ch on part)
        bqk_sb = const.tile([1, 2 * CH], BF16)
        bv_sb = const.tile([1, CH], BF16)
        ones = const.tile([1, 512], BF16)
        v4 = const.tile([P, NKC, HPC * (HD + 1)], BF16)  # v + ones column

        # DMA descriptor generation costs ~600ns per transfer and is
        # serial per issuing engine — it was the real startup bottleneck
        # (33 transfers -> ~20us of descgen on Sync alone).  Split descgen
        # across the two HW-DGE engines (Sync + ScalarE, idle at t=0),
        # first-needed transfers first in BOTH queues: v(0..3) needs
        # xt0+wv, pair-0 qk then needs wqk+xt0..3.
        # need-order per engine queue: qk half nh reads xts[nh], emitted
        # ascending, so xt1/xt2 ride sync behind xt0 while wqk+xt3 ride
        # the scalar queue behind wv.
        for k in range(KC):
            nc.sync.dma_start(xts[0][:, k, :], xT_d[k * P:(k + 1) * P, 0:512])
            nc.scalar.dma_start(wv[:, k, :], wvT_d[k * P:(k + 1) * P, :])
        for k in range(KC):
            nc.scalar.dma_start(wqk[:, k, :], wqkT_d[k * P:(k + 1) * P, :])
        for g, eng in ((1, nc.sync), (2, nc.sync), (3, nc.sync)):
            csl = slice(g * 512, (g + 1) * 512)
            for k in range(KC):
                eng.dma_start(xts[g][:, k, :], xT_d[k * P:(k + 1) * P, csl])
        for t in range(PAIRS):
            nc.sync.dma_start(wp[:, t, :], wpT_d[t * P:(t + 1) * P, :])
        nc.sync.dma_start(bqk_sb[:], bqk_d[:])
        nc.sync.dma_start(bv_sb[:], bv_d[:])
        nc.vector.memset(ones[:], 1.0)
        # Dense memset to 1.0; the v drains below only overwrite columns
        # 0..63 of each 65-wide head block, leaving column 64 == 1.0 (the
        # fused softmax-rowsum column).
        nc.vector.memset(v4[:], 1.0)
        v4r = v4.rearrange("p n (h c) -> p n h c", c=HD + 1)

        qk_tiles = {}   # t -> (qt, kt)
        at_tiles = []

        def emit_qkv_half(t, part, nh):
            """One 512-col slice of pair t's q.T or k.T (nh in 0..3).
            PSUM is allocated and drained within the call."""
            if t not in qk_tiles:
                qt_ = qkpool.tile([P, N], BF16, tag="qk", name=f"qt{t}")
                kt_ = qkpool.tile([P, N], BF16, tag="qk", name=f"kt{t}")
                qk_tiles[t] = (qt_, kt_)
            qt_, kt_ = qk_tiles[t]
            colofs = t * P if part == "q" else CH + t * P
            dst = qt_ if part == "q" else kt_
            ps = pspool.tile([P, 512], F32, tag="s", name="qkps")
            for k in range(KC):
                nc.tensor.matmul(
                    ps[:],
                    lhsT=wqk[:, k, colofs:colofs + P],
                    rhs=xts[nh][:, k, :],
                    start=(k == 0),
                    stop=(k == KC - 1 and not with_bias),
                )
            if with_bias:
                nc.tensor.matmul(
                    ps[:],
                    lhsT=bqk_sb[:, colofs:colofs + P],
                    rhs=ones[:, 0:512],
                    start=False, stop=True,
                )
            if SCHRAUDOLPH:
                # drain on ScalarE (Copy shares the Exp table, no table
                # reload): measured slightly better than DVE when the ACT
                # has slack from the exp offload
                nc.scalar.copy(dst[:, nh * 512:(nh + 1) * 512], ps[:])
            else:
                nc.vector.tensor_copy(dst[:, nh * 512:(nh + 1) * 512], ps[:])

        def emit_v(s):
            """v for all 6 heads for sequence chunk s (with fused bias)."""
            g, cofs = divmod(s, 4)
            cofs *= P
            vps = pspool.tile([P, CH], F32, tag="s", name="vps")
            for k in range(KC):
                nc.tensor.matmul(
                    vps[:],
                    lhsT=xts[g][:, k, cofs:cofs + P],
                    rhs=wv[:, k, :],
                    start=(k == 0),
                    stop=(k == KC - 1 and not with_bias),
                )
            if with_bias:
                nc.tensor.matmul(
                    vps[:], lhsT=ones[:, 0:P], rhs=bv_sb[:],
                    start=False, stop=True,
                )
            # DVE, not ScalarE: the strided 3-D destination AP (64 of every
            # 65 columns) miscompiles/NaNs through the ACT copy path
            nc.vector.tensor_copy(
                v4r[:, s, :, 0:HD],
                vps.rearrange("p (h c) -> p h c", c=HD),
            )

        def emit_proj_piece(z, drain="dve", pool=None):
            """y.T piece: dims d*128..(d+1)*128, seq s4*512..(s4+1)*512.
            drain="act" uses the ScalarE for the PSUM drain (Copy shares
            the Exp activation table, so no table reload) — only safe at
            the tail where the exp backlog is gone, since the drain holds
            a pspool slot until it executes.  The tail also alternates
            psum between pspool and upool (attention done, U slots free)
            so the drain latency never gates the next piece's matmuls."""
            s4, dchunk = divmod(z, 6)
            # pool rings are per-tag: reuse the owning pool's existing tag
            # so no new PSUM ring is reserved
            yps = (pool or pspool).tile([P, 512], F32,
                                        tag="u" if pool is upool else "s",
                                        name="yps")
            for t in range(PAIRS):
                nc.tensor.matmul(
                    yps[:],
                    lhsT=wp[:, t, dchunk * P:(dchunk + 1) * P],
                    rhs=at_tiles[t][:, s4 * 512:(s4 + 1) * 512],
                    start=(t == 0), stop=(t == PAIRS - 1),
                )
            ysb = ypool.tile([P, 512], BF16, tag="y", name="ysb")
            if drain == "act":
                nc.scalar.copy(ysb[:], yps[:])
            else:
                nc.vector.tensor_copy(ysb[:], yps[:])
            nc.sync.dma_start(
                y_d[dchunk * P:(dchunk + 1) * P, s4 * 512:(s4 + 1) * 512],
                ysb[:],
            )

        def emit_proj_wide(dchunk):
            """1024-query-wide projection piece (queries 0..1023, one
            128-dim chunk): halves the pspool ring rotations and drains
            vs two 512-wide pieces — each rotation's WAR on the exp read
            of the slot 2 allocations back stalls the PE ~0.3-0.7us AND
            drops it to the 1.2GHz pstate for ~3us."""
            yps = pspool.tile([P, QT], F32, tag="s", name="yps")
            for t in range(PAIRS):
                for n in range(2):
                    nsl = slice(n * 512, (n + 1) * 512)
                    nc.tensor.matmul(
                        yps[:, nsl],
                        lhsT=wp[:, t, dchunk * P:(dchunk + 1) * P],
                        rhs=at_tiles[t][:, nsl],
                        start=(t == 0), stop=(t == PAIRS - 1),
                    )
            ysb = ypool.tile([P, QT], BF16, tag="y", name="ysb")
            nc.vector.tensor_copy(ysb[:], yps[:])
            nc.sync.dma_start(y_d[dchunk * P:(dchunk + 1) * P, 0:QT], ysb[:])

        def emit_u(t, ua, ub, ea, eb, i):
            # same-stationary matmuls adjacent: ua n0, ua n1, ub n0, ub n1
            for dst, e, h in ((ua, ea, 2 * t), (ub, eb, 2 * t + 1)):
                for n in range(QT // 512):
                    nsl = slice(n * 512, (n + 1) * 512)
                    nc.tensor.matmul(
                        dst[:, nsl], lhsT=v4r[:, i, h, :], rhs=e[:, nsl],
                        start=(i == 0), stop=(i == NKC - 1),
                    )

        def drain_u_half(u, name):
            """Drain one U psum tile's two halves to SBUF (recycles the
            PSUM slot for the next group's accumulation)."""
            out = []
            for h in range(2):
                us = uspool.tile([HD + 1, 512], F32, tag="us", name=name)
                nc.vector.tensor_copy(us[:], u[:, h * 512:(h + 1) * 512])
                out.append(us)
            return out

        def make_norm_thunks(t, j, ua, ub, at):
            """Boundary: drain ua now; the rest becomes small DVE thunks
            (<=1.2us each) popped one per chunk, so the DVE queue convoy
            never delays the Schraudolph exp (the S psum ring has a
            distance-1 WAR on it — a late DVE stalls the PE and drops it
            out of its 2.4GHz pstate).  Per-head 512-wide pieces keep each
            thunk's DVE time minimal."""
            st = {"usa": drain_u_half(ua, "usa")}
            thunks = []

            def t_drain_b():
                st["usb"] = drain_u_half(ub, "usb")

            def t_recip(h, side):
                u_half = st["usa" if side == "a" else "usb"][h]
                rs1 = rpool.tile([1, 512], F32, tag="rs", name="rs1")
                nc.sync.dma_start(rs1[:], u_half[HD:HD + 1, :])
                rc1 = rpool.tile([1, 512], F32, tag="r", name="rc1")
                nc.vector.reciprocal_approx_fast(rc1[:], rs1[:])
                rb1 = rbpool.tile([HD, 512], F32, tag="rb", name="rb1")
                nc.gpsimd.partition_broadcast(rb1[:], rc1[:])
                st[f"rb_{side}{h}"] = rb1

            def t_mul_a(h):
                jhsl = slice(j * QT + h * 512, j * QT + (h + 1) * 512)
                nc.vector.tensor_mul(at[0:HD, jhsl], st["usa"][h][0:HD, :],
                                     st[f"rb_a{h}"][:])

            def t_mul_b(h):
                jhsl = slice(j * QT + h * 512, j * QT + (h + 1) * 512)
                stg = uspool.tile([HD, 512], BF16, tag="stg", name="stg")
                nc.vector.tensor_mul(stg[:], st["usb"][h][0:HD, :],
                                     st[f"rb_b{h}"][:])
                nc.sync.dma_start(at[HD:P, jhsl], stg[:])

            thunks.append(t_drain_b)
            for h in range(2):
                thunks.append(lambda h=h: t_recip(h, "a"))
                thunks.append(lambda h=h: t_mul_a(h))
                thunks.append(lambda h=h: t_recip(h, "b"))
                thunks.append(lambda h=h: t_mul_b(h))
            return thunks


        # ---- warmup: v(0..3), then pair-0 q/k (dense PE stream; keeping
        # the DMA-gated qkv out of the pipelined loop — an early-start
        # variant with qk halves as (0,0) fillers measured 55us WORSE:
        # mid-pipeline DMA stalls cascade through the psum rings) ---------
        for s in range(4):
            emit_v(s)
        for part in ("q", "k"):
            for nh in range(4):
                emit_qkv_half(0, part, nh)

        # filler schedule: (t, j, i) -> [thunks] emitted after that chunk's U
        filler = {}

        def add_filler(key, fn):
            filler.setdefault(key, []).append(fn)

        for s in range(4, NKC):
            add_filler((0, 0, s - 2), lambda s=s: emit_v(s))
        # pair-1 halves ride in (0,1) (j=0 hosts the v fillers); pair-2
        # halves split across (1,0) and (1,1).
        qkv_spots = {
            0: [(1, i_) for i_ in (2, 3, 5, 6, 8, 9, 11, 12)],
            1: [(j_, i_) for j_ in range(2) for i_ in (2, 5, 8, 11)],
        }
        for t_ in range(PAIRS - 1):
            pieces = [(prt, nh) for prt in ("q", "k") for nh in range(4)]
            for (prt, nh), (j_, i_) in zip(pieces, qkv_spots[t_]):
                add_filler((t_, j_, i_),
                           lambda prt=prt, nh=nh, t_=t_:
                           emit_qkv_half(t_ + 1, prt, nh))
        # proj pieces must be EMITTED after the deferred normalize thunks
        # that write the at columns they read (in-order dependency
        # tracking: a reader emitted before its writer reads stale data):
        # with 9 thunks popped from chunk 1, (2,0)'s h1 muls finish by
        # chunk 9, so the six 1024-wide pieces sit at chunks 10..15.
        # dchunk 2..5 are held back to fill the tail's normalize-chain
        # PE gap (they only read at[:, 0:1024] — no tail dependency)
        for zz in range(2):
            add_filler((2, 1, 10 + zz), lambda zz=zz: emit_proj_wide(zz))

        # ---- attention: software-pipelined, continuous across j/pair
        # boundaries.  Per chunk the PE sees [S x4][U x4][filler]; the U
        # matmuls for chunk c are emitted AFTER chunk c+1's S + exp so the
        # exp stream never waits on PSUM recycling.
        pend = None   # (t, j, ua, ub, ea, eb, i, at)
        thunks = []   # deferred normalize pieces, one popped per chunk
        for t in range(PAIRS):
            qt_, kt_ = qk_tiles[t]
            at = atpool.tile([P, N], BF16, tag="at", name=f"at{t}")
            at_tiles.append(at)
            for j in range(N // QT):
                ua = upool.tile([HD + 1, QT], F32, tag="u", name="ua")
                ub = upool.tile([HD + 1, QT], F32, tag="u", name="ub")
                for i in range(NKC):
                    sa = pspool.tile([P, QT], F32, tag="s", name="sa")
                    sb = pspool.tile([P, QT], F32, tag="s", name="sb")
                    # same-stationary matmuls adjacent (measured ~5us
                    # better than alternating heads per n-slice)
                    for dst, row in ((sa, slice(0, HD)), (sb, slice(HD, P))):
                        for n in range(QT // 512):
                            qsl = slice(j * QT + n * 512, j * QT + (n + 1) * 512)
                            nc.tensor.matmul(
                                dst[:, n * 512:(n + 1) * 512],
                                lhsT=kt_[row, i * P:(i + 1) * P],
                                rhs=qt_[row, qsl],
                                start=True, stop=True,
                            )
                    ea = epool.tile([P, QT], BF16, tag="e", name="ea")
                    nc.scalar.activation(ea[:], sa[:], EXP, scale=SCALE)
                    eb = epool.tile([P, QT], BF16, tag="e", name="eb")
                    nc.scalar.activation(eb[:, 0:512], sb[:, 0:512], EXP,
                                         scale=SCALE)
                    if SCHRAUDOLPH:
                        # DVE Schraudolph exp for the second half of head B
                        # (25% of the exp stream off the ScalarE).
                        nc.vector.tensor_scalar(
                            eb.bitcast(I16)[:, 512:QT], sb[:, 512:QT],
                            SCH_A, SCH_B,
                            mybir.AluOpType.mult, mybir.AluOpType.add)
                    else:
                        nc.scalar.activation(eb[:, 512:QT], sb[:, 512:QT],
                                             EXP, scale=SCALE)
                    # pop a deferred normalize piece BEFORE the U emission
                    # so the prev group's ub drain precedes the next U
                    # matmuls in the DVE/PE queues
                    if thunks:
                        thunks.pop(0)()
                    if pend is not None:
                        pt, pj, pua, pub, pea, peb, pi, pat = pend
                        emit_u(pt, pua, pub, pea, peb, pi)
                        if pi == NKC - 1:
                            thunks = make_norm_thunks(pt, pj, pua, pub, pat)
                    pend = (t, j, ua, ub, ea, eb, i, at)
                    for fn in filler.get((t, j, i), ()):
                        fn()
        pt, pj, pua, pub, pea, peb, pi, pat = pend
        for th in thunks:
            th()

        # ---- tail: final U matmuls with the drains interleaved per
        # query-half (the h0 drains run while the PE streams h1), then a
        # latency-minimized per-head normalize chain feeding the last 12
        # projection pieces (seq 1024..2047).
        us_t = []
        for n in range(2):
            nsl = slice(n * 512, (n + 1) * 512)
            for u_, h_ in ((pua, 2 * pt), (pub, 2 * pt + 1)):
                nc.tensor.matmul(
                    u_[:, nsl], lhsT=v4r[:, pi, h_, :], rhs=(pea, peb)[h_ % 2][:, nsl],
                    start=False, stop=True,
                )
            usa_n = uspool.tile([HD + 1, 512], F32, tag="us", name="usa")
            nc.vector.tensor_copy(usa_n[:], pua[:, nsl])
            usb_n = uspool.tile([HD + 1, 512], F32, tag="us", name="usb")
            nc.vector.tensor_copy(usb_n[:], pub[:, nsl])
            us_t.append((usa_n, usb_n))
        # reserved wide pieces fill the PE while the first normalize chain
        # (drain -> rowsum DMA -> recip -> broadcast -> mul -> shift DMA,
        # ~5.5us serial) runs on DVE/GpSimd/DMA
        for zz in range(2, 6):
            emit_proj_wide(zz)

        def tail_norm_head(u_half, rows, jhsl, to_b):
            """Per-head 512-wide chain (shorter serial latency than the
            merged 1024-wide variant used mid-kernel)."""
            rs1 = rpool.tile([1, 512], F32, tag="rs", name="rs1")
            nc.sync.dma_start(rs1[:], u_half[HD:HD + 1, :])
            rc1 = rpool.tile([1, 512], F32, tag="r", name="rc1")
            nc.vector.reciprocal_approx_fast(rc1[:], rs1[:])
            rb1 = rbpool.tile([HD, 512], F32, tag="rb", name="rb1")
            nc.gpsimd.partition_broadcast(rb1[:], rc1[:])
            if not to_b:
                nc.vector.tensor_mul(pat[0:HD, jhsl], u_half[0:HD, :], rb1[:])
            else:
                stg = uspool.tile([HD, 512], BF16, tag="stg", name="stg")
                nc.vector.tensor_mul(stg[:], u_half[0:HD, :], rb1[:])
                nc.sync.dma_start(pat[HD:P, jhsl], stg[:])

        for h in range(2):
            jhsl = slice(pj * QT + h * 512, pj * QT + (h + 1) * 512)
            tail_norm_head(us_t[h][0], slice(0, HD), jhsl, False)
            tail_norm_head(us_t[h][1], slice(HD, P), jhsl, True)
            for q, z in enumerate(range(12 + 6 * h, 18 + 6 * h)):
                emit_proj_piece(z, drain="act" if z % 2 else "dve",
                                pool=upool if q % 2 else pspool)


def build_program(with_bias=True):
    nc = bacc.Bacc(
        "TRN2", target_bir_lowering=False, debug=False, num_devices=NCORES
    )
    xT_d = nc.dram_tensor("xT", [DIM, N], BF16, kind="ExternalInput").ap()
    wqkT_d = nc.dram_tensor("wqkT", [DIM, 2 * CH], BF16, kind="ExternalInput").ap()
    wvT_d = nc.dram_tensor("wvT", [DIM, CH], BF16, kind="ExternalInput").ap()
    bqk_d = nc.dram_tensor("bqk", [1, 2 * CH], BF16, kind="ExternalInput").ap()
    bv_d = nc.dram_tensor("bv", [1, CH], BF16, kind="ExternalInput").ap()
    wpT_d = nc.dram_tensor("wpT", [CH, DIM], BF16, kind="ExternalInput").ap()
    # transposed output y.T [DIM, N] bf16; the host upcasts + transposes
    y_d = nc.dram_tensor("y", [DIM, N], BF16, kind="ExternalOutput").ap()
    with tile.TileContext(nc) as tc:
        _emit(tc, xT_d, wqkT_d, wvT_d, bqk_d, bv_d, wpT_d, y_d, with_bias)
    nc.compile()
    return nc


def get_program(with_bias=True):
    if with_bias not in _PROGRAMS:
        _PROGRAMS[with_bias] = build_program(with_bias)
    return _PROGRAMS[with_bias]


def make_in_maps(x, Wqkv, bqkv, Wproj):
    import ml_dtypes

    bf16 = ml_dtypes.bfloat16
    x = np.ascontiguousarray(np.asarray(x, np.float32))
    Wqkv = np.asarray(Wqkv, np.float32)
    bqkv = np.asarray(bqkv, np.float32)
    in_maps = []
    for c in range(NCORES):
        b, g = divmod(c, GPB)
        cs = slice(g * CH, (g + 1) * CH)
        wq = Wqkv[0 * DIM:1 * DIM][cs]
        wk = Wqkv[1 * DIM:2 * DIM][cs]
        wv_ = Wqkv[2 * DIM:3 * DIM][cs]
        in_maps.append({
            "xT": np.ascontiguousarray(x[b].T).astype(bf16),
            "wqkT": np.ascontiguousarray(
                np.concatenate([wq, wk], 0).T).astype(bf16),
            "wvT": np.ascontiguousarray(wv_.T).astype(bf16),
            "bqk": np.concatenate(
                [bqkv[0 * DIM:1 * DIM][cs], bqkv[1 * DIM:2 * DIM][cs]]
            )[None, :].astype(bf16),
            "bv": bqkv[2 * DIM:3 * DIM][cs][None, :].astype(bf16),
            "wpT": np.ascontiguousarray(
                np.asarray(Wproj, np.float32)[:, cs].T).astype(bf16),
        })
    return in_maps


def combine_outputs(per_core_y, bproj):
    bproj = np.asarray(bproj, np.float32)
    out = np.empty((B, N, DIM), np.float32)
    for b in range(B):
        out[b] = (np.asarray(per_core_y[GPB * b], np.float32)
                  + np.asarray(per_core_y[GPB * b + 1], np.float32)).T \
            + bproj[None, :]
    return out


def kernel(**inputs):
    ratio = int(np.asarray(inputs.get("ratio", 1)))
    assert ratio == 1, f"kernel specialized for ratio=1, got {ratio}"
    with_bias = bool(np.any(np.asarray(inputs["bqkv"], np.float32)))
    nc = get_program(with_bias)
    in_maps = make_in_maps(
        inputs["x"], inputs["Wqkv"], inputs["bqkv"], inputs["Wproj"]
    )
    res = run_bass_kernel_spmd(nc, in_maps, list(range(NCORES)))
    ys = [res.results[c]["y"] for c in range(NCORES)]
    return combine_outputs(ys, inputs["bproj"])
